# revision 2
# baseline (speedup 1.0000x reference)
"""Trainium2 Bass kernel for vq_codebook (nn_HI4B1C_codebook).

reference semantics:
    scores = 2*(X @ grid.T) - grid_norm   # (N, 16)
    idx  = argmax(scores, -1)             # nearest codeword
    vals = grid[idx]                      # (N, 1)

The grid is the fixed half-integer lattice arange(-8, 8) + 0.5, so the
nearest codeword of x is idx = rint(x + 7.5) (clipped to [0, 15]) and
vals = idx - 7.5.  rint is computed with the fp32 magic-number trick:
t = (x + 7.5) + 1.5*2^23 rounds the fractional bits away; t - 1.5*2^23
is then the exact integer rint(x+7.5).

Sharding: data-parallel over N across 8 NeuronCores; each core handles
N/8 = 1,048,576 elements laid out as SBUF-friendly (128, 8192).

Per 2048-wide chunk (per core):
  DMA in  x                     (1 MiB)
  DVE     t    = (x + 7.5) + MAGIC        one dual-op tensor_scalar pass
  ACT     idx  = uint8(t - MAGIC)         activation Identity, bias=-MAGIC
  DVE     vals = (t - MAGIC) - 7.5        one dual-op tensor_scalar pass
  DMA out vals (1 MiB), idx (0.25 MiB)

Traffic is 9 MiB/core vs ~3 fp32 element passes -> memory-bound; the
kernel pipelines chunks with bufs=3 so DMA stays saturated.
"""

import numpy as np

import concourse.bacc as bacc
import concourse.mybir as mybir
from concourse.tile import TileContext
from concourse.bass_utils import run_bass_kernel_spmd

N = 8_388_608
N_CORES = 8
P = 128
FD_CORE = N // N_CORES // P  # 8192 fp32 elements per partition per core
FD_TILE = 2048
MAGIC = 12582912.0  # 1.5 * 2**23

_NC_CACHE = {}


def build_nc(fd_tile=FD_TILE, bufs=3):
    nc = bacc.Bacc("TRN2")
    f32 = mybir.dt.float32
    u8 = mybir.dt.uint8
    x = nc.dram_tensor("x", [P, FD_CORE], f32, kind="ExternalInput")
    vals = nc.dram_tensor("vals", [P, FD_CORE], f32, kind="ExternalOutput")
    idx = nc.dram_tensor("idx", [P, FD_CORE], u8, kind="ExternalOutput")

    with TileContext(nc) as tc:
        with tc.tile_pool(name="const", bufs=1) as cpool, \
             tc.tile_pool(name="work", bufs=bufs) as pool:
            nbias = cpool.tile([P, 1], f32)
            nc.vector.memset(nbias[:], -MAGIC)
            for c in range(FD_CORE // fd_tile):
                sl = slice(c * fd_tile, (c + 1) * fd_tile)
                xt = pool.tile([P, fd_tile], f32, tag="xt")
                nc.sync.dma_start(out=xt[:], in_=x[:, sl])
                tt = pool.tile([P, fd_tile], f32, tag="tt")
                nc.vector.tensor_scalar(
                    out=tt[:], in0=xt[:], scalar1=7.5, scalar2=MAGIC,
                    op0=mybir.AluOpType.add, op1=mybir.AluOpType.add,
                )
                it = pool.tile([P, fd_tile], u8, tag="it")
                nc.scalar.activation(
                    out=it[:], in_=tt[:],
                    func=mybir.ActivationFunctionType.Identity,
                    bias=nbias[:], scale=1.0,
                )
                vt = pool.tile([P, fd_tile], f32, tag="vt")
                nc.vector.tensor_scalar(
                    out=vt[:], in0=tt[:], scalar1=MAGIC, scalar2=7.5,
                    op0=mybir.AluOpType.subtract, op1=mybir.AluOpType.subtract,
                )
                nc.sync.dma_start(out=vals[:, sl], in_=vt[:])
                nc.sync.dma_start(out=idx[:, sl], in_=it[:])
    nc.compile()
    return nc


def _get_nc():
    if "nc" not in _NC_CACHE:
        try:
            import kernel_raw
            _NC_CACHE["nc"] = kernel_raw.build_nc()
        except Exception:
            _NC_CACHE["nc"] = build_nc()
    return _NC_CACHE["nc"]


def run(X, trace=False, **spmd_kwargs):
    """Run the SPMD kernel on full input X (N,1) f32; returns
    (vals (N,1) f32, idx (N,) u8, BassKernelResults)."""
    X = np.ascontiguousarray(np.asarray(X, dtype=np.float32)).reshape(
        N_CORES, P, FD_CORE
    )
    nc = _get_nc()
    in_maps = [{"x": X[c]} for c in range(N_CORES)]
    res = run_bass_kernel_spmd(
        nc, in_maps, core_ids=list(range(N_CORES)), trace=trace, **spmd_kwargs
    )
    vals = np.concatenate(
        [r["vals"].reshape(-1) for r in res.results]
    ).reshape(N, 1)
    idx = np.concatenate([r["idx"].reshape(-1) for r in res.results])
    return vals, idx.astype(np.uint8), res


def kernel(X, grid=None, grid_norm=None):
    vals, idx, _ = run(X, trace=False)
    return vals, idx
